# revision 9
# baseline (speedup 1.0000x reference)
"""Cost-volume concat kernel for Trainium2 (8 NeuronCores, SPMD).

Problem: left/right (B=4, C=32, H=64, W=128) f32 ->
         out (B, 2C, D=48, H, W) where
  out[b, c,    d, h, w] = left [b, c, h, w]     * (w >= d)
  out[b, C+c,  d, h, w] = right[b, c, h, w - d] * (w >= d)

Sharding: 8 cores = 4 batches x 2 disparity-halves (d0 in {0, 24}).
All cores run an IDENTICAL program (single SPMD NEFF); the d0 shift is
absorbed host-side by pre-shifting the left input by d0 columns and
stitching the per-core output back with a d0 column offset:

  core (b, q), d0 = 24q, level i in [0, 24):
    xl[c,h,w]      = left[b,c,h,w+d0]  (zero-padded tail)
    xr[c,h,24+w]   = right[b,c,h,w]    (24 leading zero columns baked in)
    yl[c, i, h, w] = xl[c,h,w] * (w >= i)
    yr[c, i, h, w] = xr[c,h,w-i] * (w >= i)
  host: out[b, 0:C, d0+i, h, d0+w] = yl[c, i, h, w]
        out[b, C:,  d0+i, h, d0+w] = yr[c, i, h, w]   (rest stays zero)

The kernel is pure DMA (no compute):
  - right half: full-width sliding-window reads from the padded tile
    (the pad supplies the w < i zeros), 24 x 1MB stores;
  - left half: the w >= i tail only -- output buffers are zero-filled
    by the runtime (run_bass_kernel_spmd pre-zeros ExternalOutputs on
    both the native and the PJRT/axon path), so masked zeros need no
    write at all;
  - every DMA carries at most one sync wait (walrus's HWDGE direct2d
    limit): data deps exist only against the two input loads, which the
    first DMA of each ring observes once.
"""

import sys

for _p in ("/opt/trn_rl_repo",):
    if _p not in sys.path:
        sys.path.append(_p)

import numpy as np

import concourse.bass as bass
import concourse.mybir as mybir
import concourse.tile as tile
from concourse.bass_utils import run_bass_kernel_spmd

B, C, H, W = 4, 32, 64, 128
D = 48
NCORES = 8
DL = D // 2          # 24 disparity levels per core
PAD = DL             # zero-pad columns for the shifted right-half reads
ROWS = C * H // 128  # 16 (c,h)-rows per SBUF partition

_F32 = mybir.dt.float32

_NC_CACHE = {}


class _SplitDrainTC(tile.TileContext):
    """TileContext whose kernel-tail drain legalizes to <=1 sem wait per
    instruction: this walrus pipeline (policy 0, no sync passes) rejects
    any instruction carrying more than one sync wait, and the stock
    _drain_and_barrier puts every outstanding DMA-lane sem on one Drain.
    We keep the first wait on the drain and chain the rest through extra
    single-wait drains on the same (in-order) SP queue."""

    def _drain_and_barrier(self, tick_clock, wait_clock):
        from concourse.vector_clock import ScopedClock

        nc = self.nc
        drain_inst = nc.sync.drain(fusable=False)
        wait_clock.add_sem_waits(
            drain_inst.ins, ScopedClock({None: tick_clock.global_clock})
        )
        si = drain_inst.ins.sync_info
        if si is not None and len(si.on_wait) > 1:
            waits = list(si.on_wait)
            drain_inst.ins.sync_info = mybir.SyncInfo(
                on_wait=[waits[0]], on_update=list(si.on_update)
            )
            for w in waits[1:]:
                extra = nc.sync.drain(fusable=False)
                extra.ins.sync_info = mybir.SyncInfo(on_wait=[w], on_update=[])

        nc.all_engine_barrier()
        assert self.sems is not None
        popped = nc._tile_sem_poison_stack.pop()
        assert popped is self._sem_poison
        nc.clear_and_free_semaphores(list(self.sems.allocated().values()))
        nc.all_engine_barrier()


def _build_nc():
    """One SPMD program for every core; ~52 instructions, no control flow."""
    nc = bass.Bass()
    xl = nc.dram_tensor("xl", [C, H, W], _F32, kind="ExternalInput")
    xr = nc.dram_tensor("xr", [C, H, PAD + W], _F32, kind="ExternalInput")
    # Two outputs, one per HWDGE ring: a single shared output tensor makes
    # Tile emit cross-engine WAW waits on every DMA (walrus rejects >1 sync
    # wait per HWDGE DMA); disjoint tensors keep each ring's DMAs dep-free.
    yl = nc.dram_tensor("yl", [C, DL, H, W], _F32, kind="ExternalOutput")
    yr = nc.dram_tensor("yr", [C, DL, H, W], _F32, kind="ExternalOutput")

    with _SplitDrainTC(nc) as tc:
        with tc.tile_pool(name="pool", bufs=1) as pool:
            # Partition p holds 16 consecutive (c,h) rows -> every DMA AP
            # collapses to <=3 dims with contiguous inner runs.
            lt = pool.tile([128, ROWS, W], _F32, name="lt")
            rt = pool.tile([128, ROWS, PAD + W], _F32, name="rt")

            # Loads ride the same two HWDGE rings as the stores: SWDGE lanes
            # would add two more sems to the kernel-tail drain, which only
            # supports 8 sync waits.
            nc.sync.dma_start(lt[:], xl[:])
            nc.scalar.dma_start(rt[:], xr[:])

            for i in range(DL):
                # Right half (ACT ring): full 512B rows; the window start
                # walks back through the pad, which supplies the zeros.
                nc.scalar.dma_start(
                    yr[:, i, :, :], rt[:, :, PAD - i:PAD - i + W]
                )
                # Left half (SP ring): only the unmasked w >= i tail; the
                # pre-zeroed output keeps the masked prefix at zero.
                if i == 0:
                    nc.sync.dma_start(yl[:, 0, :, :], lt[:])
                else:
                    nc.sync.dma_start(yl[:, i, :, i:], lt[:, :, i:])
    return nc


def _get_nc():
    if "nc" not in _NC_CACHE:
        _NC_CACHE["nc"] = _build_nc()
    return _NC_CACHE["nc"]


def _run(left, right, **spmd_kwargs):
    left = np.ascontiguousarray(np.asarray(left), dtype=np.float32)
    right = np.ascontiguousarray(np.asarray(right), dtype=np.float32)

    in_maps = []
    for k in range(NCORES):
        b, q = divmod(k, 2)
        d0 = DL * q
        xl = np.zeros((C, H, W), np.float32)
        xl[:, :, :W - d0] = left[b, :, :, d0:]
        xr = np.zeros((C, H, PAD + W), np.float32)
        xr[:, :, PAD:] = right[b]
        in_maps.append({"xl": xl, "xr": xr})

    res = run_bass_kernel_spmd(
        _get_nc(), in_maps, core_ids=list(range(NCORES)), **spmd_kwargs
    )

    out = np.zeros((B, 2 * C, D, H, W), np.float32)
    for k in range(NCORES):
        b, q = divmod(k, 2)
        d0 = DL * q
        out[b, 0:C, d0:d0 + DL, :, d0:] = res.results[k]["yl"][:, :, :, :W - d0]
        out[b, C:, d0:d0 + DL, :, d0:] = res.results[k]["yr"][:, :, :, :W - d0]
    return out, res


def kernel(left, right):
    out, _ = _run(left, right)
    return out
